# revision 1
# baseline (speedup 1.0000x reference)
"""Trainium2 Bass kernel for nn_BitNodeTrellis.

res[b,n,u,i,j] = logsumexp_{s}( e1[b,n,(u+uhat[b,n])%2,i,s] + e2[b,n,u,s,j] )

Full shapes: e1,e2 [256, 8192, 2, 2, 2] f32, uhat [256, 8192] int32.
Fully data-parallel over B1=256: each of the 8 NeuronCores gets 32 codewords
(ROWS = 32*8192 = 262144 independent rows of 8 channels).

Math per row, in exp domain (LSE == log of a 2x2 matmul of exponentials):
    EA = exp(e1), EB = exp(e2)
    EA' = u-swap of EA where uhat == 1   (select commutes with exp)
    r[u,i,j] = EA'[u,i,0]*EB[u,0,j] + EA'[u,i,1]*EB[u,1,j]
    out = log(r)

On-chip layout: rows tiled as [128 partitions, ft rows, 8 channels]
(channels fastest => contiguous DMA). Channel arithmetic uses strided /
broadcast access patterns so each instruction covers whole channel groups:

  once:  DMA uhat; POOL expands it to a 4-wide f32 mask (stride-5 pitch)
  tile:  DMA e1,e2 | ACT exp(a) exp(b), t3=copy(EA_lo)
         DVE copy_predicated x2 (u-swap), mul P0, mul P1, add
         ACT ln | DMA out

The per-tile row counts taper at the ends to shorten pipeline fill/drain.
A single activation-table set (natural_log_exp_and_others) covers
Exp/Ln/Copy, so the compiled program loads the ACT LUT exactly once.
"""

import numpy as np

import concourse.bass as bass
import concourse.bacc as bacc
import concourse.mybir as mybir
import concourse.tile as tile
from concourse.bass_utils import run_bass_kernel_spmd

F32 = mybir.dt.float32
I32 = mybir.dt.int32

P = 128
ACT = mybir.ActivationFunctionType

B1, B2 = 256, 8192
NCORES = 8
B1_SH = B1 // NCORES                  # 32 codewords per core
ROWS = B1_SH * B2                     # 262144 rows per core
RPP = ROWS // P                       # 2048 rows per partition
FTS = [64, 96, 160, 224, 256, 256, 256, 256, 224, 160, 96]  # sums to 2048

COMBINED_ACT_TABLE = "natural_log_exp_and_others"


class _combined_act_table:
    """Constrain bacc's activation-table chooser to the one real table set
    that contains Exp, Ln and Copy, so it emits a single LoadActFuncSet
    instead of reloading the LUT on every Exp<->Ln alternation. The emitted
    act_func_set_id still indexes the genuine act_info.json entry."""

    def __enter__(self):
        self._orig = bacc.get_activation_tables
        orig = self._orig

        def constrained(arch):
            tabs = orig(arch)
            need = {ACT.Exp, ACT.Ln, ACT.Copy}
            if not need.issubset(tabs.get(COMBINED_ACT_TABLE, set())):
                return tabs  # unexpected act_info: leave untouched
            return {
                name: (s if name == COMBINED_ACT_TABLE else set())
                for name, s in tabs.items()
            }

        bacc.get_activation_tables = constrained

    def __exit__(self, *a):
        bacc.get_activation_tables = self._orig


def build_program(rows=ROWS, fts=None, repeat=1):
    rpp = rows // P
    if fts is None:
        fts = [rpp // 8] * 8
    assert sum(fts) == rpp and rows % P == 0
    ftmax = max(fts)

    nc = bacc.Bacc(
        "TRN2",
        target_bir_lowering=False,
        debug=False,
        num_devices=NCORES,
    )

    e1_d = nc.dram_tensor("e1", [P, rpp * 8], F32, kind="ExternalInput").ap()
    e2_d = nc.dram_tensor("e2", [P, rpp * 8], F32, kind="ExternalInput").ap()
    uh_d = nc.dram_tensor("uhat", [P, rpp], I32, kind="ExternalInput").ap()
    out_d = nc.dram_tensor("out", [P, rpp * 8], F32, kind="ExternalOutput").ap()

    def body(tc):
        with (
            tc.tile_pool(name="stat", bufs=1) as stat,
            tc.tile_pool(name="inp", bufs=3) as inp,
            tc.tile_pool(name="scr", bufs=3) as scr,
            tc.tile_pool(name="outp", bufs=3) as outp,
        ):
            uall = stat.tile([P, rpp], I32, tag="uall")
            w4all = stat.tile([P, rpp * 5], I32, tag="w4all")
            w4v = w4all[:].rearrange("p (f c) -> p f c", c=5)[:, :, 0:4]

            # uhat DMA in two pieces (tile-0 chunk first); mask expansion
            # chunked per tile so tile 0's select is ready early
            nc.sync.dma_start(uall[:, : fts[0]], uh_d[:, : fts[0]])
            nc.sync.dma_start(uall[:, fts[0] :], uh_d[:, fts[0] :])
            f0 = 0
            for ft in fts:
                ub = uall[:, f0 : f0 + ft].unsqueeze(2).broadcast_to([P, ft, 4])
                nc.gpsimd.tensor_copy(w4v[:, f0 : f0 + ft, :], ub)
                f0 += ft

            f0 = 0
            for ft in fts:
                a_t = inp.tile([P, ftmax * 8], F32, tag="a")
                b_t = inp.tile([P, ftmax * 8], F32, tag="b")
                a = a_t[:, : ft * 8]
                b = b_t[:, : ft * 8]
                nc.sync.dma_start(a, e1_d[:, f0 * 8 : (f0 + ft) * 8])
                nc.sync.dma_start(b, e2_d[:, f0 * 8 : (f0 + ft) * 8])

                tmp = scr.tile([P, ftmax * 5], F32, tag="tmp")
                r2_t = scr.tile([P, ftmax * 8], F32, tag="r2")
                r_t = outp.tile([P, ftmax * 8], F32, tag="r")
                r2 = r2_t[:, : ft * 8]
                r = r_t[:, : ft * 8]

                nc.scalar.activation(a, a, ACT.Exp)
                nc.scalar.activation(b, b, ACT.Exp)

                a3 = a.rearrange("p (f c) -> p f c", c=8)
                t3 = tmp[:].rearrange("p (f c) -> p f c", c=5)[:, :ft, 0:4]
                w43 = w4v[:, f0 : f0 + ft, :]

                nc.scalar.activation(t3, a3[:, :, 0:4], ACT.Copy)

                nc.vector.copy_predicated(a3[:, :, 0:4], w43, a3[:, :, 4:8])
                nc.vector.copy_predicated(a3[:, :, 4:8], w43, t3)

                ea = a.rearrange("p (f u i s) -> p f u i s", u=2, i=2, s=2)
                eb = b.rearrange("p (f u s j) -> p f u s j", u=2, s=2, j=2)
                r4 = r.rearrange("p (f u i j) -> p f u i j", u=2, i=2, j=2)
                r24 = r2.rearrange("p (f u i j) -> p f u i j", u=2, i=2, j=2)

                ea0 = ea[:, :, :, :, 0].unsqueeze(4).broadcast_to([P, ft, 2, 2, 2])
                ea1 = ea[:, :, :, :, 1].unsqueeze(4).broadcast_to([P, ft, 2, 2, 2])
                eb0 = eb[:, :, :, 0, :].unsqueeze(3).broadcast_to([P, ft, 2, 2, 2])
                eb1 = eb[:, :, :, 1, :].unsqueeze(3).broadcast_to([P, ft, 2, 2, 2])

                nc.vector.tensor_mul(r4, ea0, eb0)
                nc.vector.tensor_mul(r24, ea1, eb1)
                nc.vector.tensor_add(r, r, r2)

                nc.scalar.activation(r, r, ACT.Ln)

                nc.sync.dma_start(out_d[:, f0 * 8 : (f0 + ft) * 8], r)
                f0 += ft

    with _combined_act_table():
        with tile.TileContext(nc) as tc:
            if repeat == 1:
                body(tc)
            else:
                with tc.For_i(0, repeat, 1):
                    body(tc)
        nc.compile()
    return nc


_NC_CACHE = {}


def _get_nc():
    if "nc" not in _NC_CACHE:
        _NC_CACHE["nc"] = build_program(fts=FTS)
    return _NC_CACHE["nc"]


def _shard(arr, c):
    return np.ascontiguousarray(arr[c * B1_SH : (c + 1) * B1_SH])


def make_in_maps(e1, e2, uhat):
    e1 = np.ascontiguousarray(e1, dtype=np.float32)
    e2 = np.ascontiguousarray(e2, dtype=np.float32)
    uhat = np.ascontiguousarray(uhat, dtype=np.int32)
    in_maps = []
    for c in range(NCORES):
        in_maps.append(
            {
                "e1": _shard(e1, c).reshape(P, RPP * 8),
                "e2": _shard(e2, c).reshape(P, RPP * 8),
                "uhat": _shard(uhat, c).reshape(P, RPP),
            }
        )
    return in_maps


def kernel(e1: np.ndarray, e2: np.ndarray, uhat: np.ndarray) -> np.ndarray:
    nc = _get_nc()
    in_maps = make_in_maps(e1, e2, uhat)
    res = run_bass_kernel_spmd(nc, in_maps, list(range(NCORES)))
    out = np.empty((B1, B2, 2, 2, 2), dtype=np.float32)
    for c in range(NCORES):
        out[c * B1_SH : (c + 1) * B1_SH] = (
            res.results[c]["out"].reshape(B1_SH, B2, 2, 2, 2)
        )
    return out

